# revision 37
# baseline (speedup 1.0000x reference)
"""Trainium2 Bass kernel for nn_Attention_45303315038988.

  q = p @ Wh.T (+bh) ; k = r @ Wl.T + bl ; v = p @ Wg.T + bg     [N, D]
  scores = q @ k.T ; attn = softmax(scores, axis=0) ; out = p + attn @ v

Design (8 NeuronCores, sequence-parallel over the query/row axis):
  - Host pre-transposes shards to feature-major fp16 (pT, rT, W^T) so every
    matmul contracts over the SBUF partition axis; no on-device transposes.
    bh is dropped: it only shifts scores by a per-key constant, which the
    softmax over the query axis cancels exactly.
  - Phase A: each core computes its shard of k^T ([d, j], bias via
    per-partition ACT bias) and v ([j, d], bias via a K=1 ones-row matmul);
    both AllGathered in fp16. AG(k^T) is kicked first; the first K^T block
    is prefetched on the gpsimd queue ahead of AG(v); the first V half-tiles
    and the residual rows ride the HWDGE (sync) ring, which the collectives
    never block. psA triple-buffers so ACT evictions keep off the PE path.
  - Phase C: scores^T = K^T.T @ q^T puts the softmax axis on the free dim:
    per-key max is a DVE reduction; ACT computes E = exp(s - m_local) into
    fp16 SBUF (16 MB) and its accum_out produces the per-key sums for free.
    Next-shard K^T streams on the scalar HWDGE ring, prefetched one shard
    ahead (no Q7/SWDGE descriptor-gen, no collective queue-ordering).
  - Softmax globalization: (max,sum) stats are AllGathered in two halves
    (the first hides under phase C) and combined locally into
    f = exp(m_local - M)/S per key. E is never rescaled.
  - Phase E: out = E^T.T @ (f*V) + p in two D-half passes with 8
    single-bank PSUM accumulators. V streams [128,512] fp16 half-tiles on
    the sync ring; ACT folds f into each tile (per-partition scale) right
    before use. The residual p enters PSUM via an fp16 identity matmul, so
    the tail is only ACT/DVE-alternating evictions that pipeline with the
    last matmuls.
All matmul operands are fp16 with fp32 PSUM accumulation; softmax
statistics are fp32. Measured rel-to-absmax error ~2.6e-3 (fp64 reference).
TimelineSim (no-collective cost model): 558 us/iter vs 575 us for the
previous i-half-pass layout.
"""
import numpy as np

P = 128
D = 1024
N = 8192
NCORES = 8
NL = N // NCORES
DB = D // P
JBL = NL // P
NG = N // P
IB = NL // P
FH = 512
VPRE = 5


def build_nc(k_iters: int = 1, no_cc: bool = False, phases: str = "full",
             opts: dict | None = None, spin_us: int = 0):
    opts = opts or {}
    import concourse.mybir as mybir
    import concourse.tile as tile
    from concourse import bacc

    f16 = mybir.dt.float16
    f32 = mybir.dt.float32
    AF = mybir.ActivationFunctionType
    AX = mybir.AxisListType
    ALU = mybir.AluOpType
    RG = [list(range(NCORES))]

    nc = bacc.Bacc("TRN2", target_bir_lowering=False, debug=False,
                   num_devices=1 if no_cc else NCORES)

    def collective(kind, op, ins, outs):
        if no_cc:
            src_ap, dst_ap = ins[0], outs[0]
            nc.sync.dma_start(out=dst_ap[0] if kind == "AllGather" else dst_ap[:],
                              in_=src_ap[:])
        else:
            nc.gpsimd.collective_compute(kind, op, replica_groups=RG,
                                         ins=[ins[0].opt()], outs=[outs[0].opt()])

    pT_h = nc.dram_tensor("pT", [D, NL], f16, kind="ExternalInput")
    rT_h = nc.dram_tensor("rT", [D, NL], f16, kind="ExternalInput")
    pres_h = nc.dram_tensor("pres", [NL, D], f16, kind="ExternalInput")
    ident_h = nc.dram_tensor("ident", [P, P], f16, kind="ExternalInput")
    WhT_h = nc.dram_tensor("WhT", [D, D], f16, kind="ExternalInput")
    WlT_h = nc.dram_tensor("WlT", [D, D], f16, kind="ExternalInput")
    WgT_h = nc.dram_tensor("WgT", [D, D], f16, kind="ExternalInput")
    bl_h = nc.dram_tensor("bl_r", [P, DB], f32, kind="ExternalInput")
    bg_h = nc.dram_tensor("bg16", [1, D], f16, kind="ExternalInput")
    ones_h = nc.dram_tensor("ones16", [1, P], f16, kind="ExternalInput")
    out_h = nc.dram_tensor("out", [NL, D], f32, kind="ExternalOutput")

    with tile.TileContext(nc) as tc:
        with tc.tile_pool(name="dram", bufs=1, space="DRAM") as dpool:
            for it in range(k_iters):
                cc_kt_in = dpool.tile([D, NL], f16, name=f"cc_kt_in{it}")
                cc_kt_out = dpool.tile([NCORES, D, NL], f16,
                                       addr_space="Shared", name=f"cc_kt_out{it}")
                cc_v_in = dpool.tile([NL, D], f16, name=f"cc_v_in{it}")
                cc_v_out = dpool.tile([NCORES, NL, D], f16,
                                      addr_space="Shared", name=f"cc_v_out{it}")
                cc_st_in = [dpool.tile([P, NG], f32, name=f"cc_st_in{it}_{h}")
                            for h in range(2)]
                cc_st_out = [dpool.tile([NCORES, P, NG], f32, addr_space="Shared",
                                        name=f"cc_st_out{it}_{h}")
                             for h in range(2)]

                with tc.tile_pool(name="lp", bufs=1) as lp:
                    qT = lp.tile([P, DB, NL], f16)
                    stats = lp.tile([P, 2, NG], f32)   # [:,0,:]=-max, [:,1,:]=sum
                    f_sc = lp.tile([P, NG], f32)
                    bl_sb = lp.tile([P, DB], f32)
                    bg_sb = lp.tile([1, D], f16)
                    ones_sb = lp.tile([1, P], f16)
                    ident_sb = lp.tile([P, P], f16)
                    nc.sync.dma_start(out=bl_sb, in_=bl_h.ap())
                    nc.sync.dma_start(out=bg_sb, in_=bg_h.ap())
                    nc.sync.dma_start(out=ones_sb, in_=ones_h.ap())
                    nc.sync.dma_start(out=ident_sb, in_=ident_h.ap())

                    if phases.startswith("C"):
                        # scores-loop microbench: fake qT/kt from inputs
                        ep_cm = tc.tile_pool(name="ep", bufs=1)
                        ep = ep_cm.__enter__()
                        E = ep.tile([P, NG, NL], f16)
                        with (
                            tc.tile_pool(name="ktp", bufs=2) as ktp,
                            tc.tile_pool(name="psC", bufs=3, space="PSUM") as psC,
                        ):
                            for db in range(DB):
                                nc.gpsimd.dma_start(
                                    out=qT[:, db, :],
                                    in_=pT_h.ap()[db * P:(db + 1) * P, :])
                            for c_idx in range(NCORES):
                                kt_c = ktp.tile([P, DB, JBL, P], f16, tag="kt")
                                for db in range(DB):
                                    nc.gpsimd.dma_start(
                                        out=kt_c[:, db, :, :].rearrange(
                                            "p a b -> p (a b)"),
                                        in_=WlT_h.ap()[db * P:(db + 1) * P, :])
                                for jlb in range(JBL):
                                    g = c_idx * JBL + jlb
                                    ps_t = psC.tile([P, NL], f32, tag="sc")
                                    for db in range(DB):
                                        for ih in range(2):
                                            nc.tensor.matmul(
                                                ps_t[:, ih * FH:(ih + 1) * FH],
                                                lhsT=kt_c[:, 0, 0, :] if phases == "C6"
                                                else kt_c[:, db, jlb, :],
                                                rhs=qT[:, db, ih * FH:(ih + 1) * FH],
                                                start=(db == 0), stop=(db == DB - 1))
                                    if phases in ("C2", "C3", "C4"):
                                        nc.vector.tensor_reduce(
                                            out=stats[:, 0, g:g + 1], in_=ps_t,
                                            op=ALU.max, axis=AX.X, negate=True)
                                    if phases == "C3":
                                        nc.scalar.activation(
                                            out=E[:, g, :], in_=ps_t, func=AF.Exp,
                                            bias=stats[:, 0, g:g + 1], scale=1.0,
                                            accum_out=stats[:, 1, g:g + 1])
                                    if phases == "C4":
                                        nc.scalar.activation(
                                            out=E[:, g, :], in_=ps_t, func=AF.Exp,
                                            bias=stats[:, 0, g:g + 1], scale=1.0)
                                    if phases == "C5":
                                        nc.scalar.activation(
                                            out=E[:, g, :], in_=ps_t, func=AF.Exp)
                            if phases in ("C1", "C6"):
                                pe_t = ktp.tile([P, NL], f32, tag="pe2")
                                nc.vector.tensor_copy(out=pe_t, in_=ps_t)
                                nc.sync.dma_start(out=out_h.ap()[0:P, :], in_=pe_t)
                            elif phases == "C3":
                                nc.sync.dma_start(out=out_h.ap()[0:P, 0:2 * NG],
                                    in_=stats.rearrange("p a b -> p (a b)"))
                            elif phases == "C4":
                                nc.sync.dma_start(out=out_h.ap()[0:P, 0:NG],
                                    in_=stats[:, 0, :])
                            else:
                                pe_t = ktp.tile([P, NL], f32, tag="pe2")
                                nc.vector.tensor_copy(out=pe_t, in_=E[:, NG - 1, :])
                                nc.sync.dma_start(out=out_h.ap()[0:P, :], in_=pe_t)
                        ep_cm.__exit__(None, None, None)
                        continue

                    # ---------------- phase A: projections ----------------
                    kt_pre = lp.tile([P, DB, JBL, P], f16, name="kt_pre")
                    vtp_cm = tc.tile_pool(name="vtp", bufs=VPRE + 2)
                    vtp = vtp_cm.__enter__()
                    with (
                        tc.tile_pool(name="pw", bufs=1) as pw,
                        tc.tile_pool(name="pst", bufs=3) as pst,
                        tc.tile_pool(name="psA", bufs=3, space="PSUM") as psA,
                    ):
                        WhT_sb = pw.tile([P, DB, D], f16)
                        WlT_sb = pw.tile([P, DB, D], f16)
                        WgT_sb = pw.tile([P, DB, D], f16)
                        pT_sb = pw.tile([P, DB, NL], f16)
                        rT_sb = pw.tile([P, DB, NL], f16)
                        # all phase-A loads ride the scalar HWDGE ring in
                        # consumption order (k-proj, then v-proj, then q-proj):
                        # RTL descriptor-gen, no Q7/SWDGE serialization, and
                        # the gpsimd queue stays free for the collectives
                        for db in range(DB):
                            for t_sb, t_h in ((WlT_sb, WlT_h), (rT_sb, rT_h)):
                                nc.scalar.dma_start(
                                    out=t_sb[:, db, :],
                                    in_=t_h.ap()[db * P:(db + 1) * P, :])
                        # v-proj consumes pT (lhsT) + WgT (rhs) next, then
                        # q-proj needs WhT: interleave in consumption order on
                        # the idle scalar HWDGE ring (no Q7 descriptor-gen),
                        # so the SWDGE queue only paces the k-proj loads
                        for db in range(DB):
                            for t_sb, t_h in ((pT_sb, pT_h), (WgT_sb, WgT_h)):
                                nc.scalar.dma_start(
                                    out=t_sb[:, db, :],
                                    in_=t_h.ap()[db * P:(db + 1) * P, :])
                        for db in range(DB):
                            nc.scalar.dma_start(
                                out=WhT_sb[:, db, :],
                                in_=WhT_h.ap()[db * P:(db + 1) * P, :])

                        # k^T shard = Wl^T.T @ r^T -> [do, j_l], + bl
                        for dob in range(DB):
                            ps_t = psA.tile([P, NL], f32)
                            for db in range(DB):
                                for ih in range(2):
                                    nc.tensor.matmul(
                                        ps_t[:, ih * FH:(ih + 1) * FH],
                                        lhsT=WlT_sb[:, db, dob * P:(dob + 1) * P],
                                        rhs=rT_sb[:, db, ih * FH:(ih + 1) * FH],
                                        start=(db == 0), stop=(db == DB - 1))
                            st = pst.tile([P, NL], f16, tag="st")
                            nc.scalar.activation(out=st, in_=ps_t, func=AF.Identity,
                                                 bias=bl_sb[:, dob:dob + 1], scale=1.0)
                            nc.sync.dma_start(out=cc_kt_in[dob * P:(dob + 1) * P, :],
                                              in_=st)
                        collective("AllGather", ALU.bypass, [cc_kt_in], [cc_kt_out])
                        # prefetch first K^T block ahead of AG(v) on gpsimd
                        for db in range(DB):
                            nc.gpsimd.dma_start(
                                out=kt_pre[:, db, :, :].rearrange("p a b -> p (a b)"),
                                in_=cc_kt_out[0, db * P:(db + 1) * P, :])

                        # v shard = p^T.T @ Wg^T -> [j_l, dv], + bg via ones-row
                        for jb in range(JBL):
                            ps_t = psA.tile([P, NL], f32)
                            for db in range(DB):
                                for dvh in range(2):
                                    nc.tensor.matmul(
                                        ps_t[:, dvh * FH:(dvh + 1) * FH],
                                        lhsT=pT_sb[:, db, jb * P:(jb + 1) * P],
                                        rhs=WgT_sb[:, db, dvh * FH:(dvh + 1) * FH],
                                        start=(db == 0), stop=False)
                            for dvh in range(2):
                                nc.tensor.matmul(
                                    ps_t[:, dvh * FH:(dvh + 1) * FH],
                                    lhsT=ones_sb[:, :],
                                    rhs=bg_sb[:, dvh * FH:(dvh + 1) * FH],
                                    start=False, stop=True)
                            st = pst.tile([P, NL], f16, tag="st")
                            nc.scalar.activation(out=st, in_=ps_t, func=AF.Copy)
                            nc.sync.dma_start(out=cc_v_in[jb * P:(jb + 1) * P, :],
                                              in_=st)
                        collective("AllGather", ALU.bypass, [cc_v_in], [cc_v_out])
                        # first phase-E V half-tiles prefetch on the HWDGE
                        # (sync) ring: their only dep is the AG(v) sem, so they
                        # stream during phase C and the stats collectives on
                        # the gpsimd ring never block them.
                        vt_pre = []
                        for g in range(VPRE):
                            c_idx, jlb = divmod(g, JBL)
                            vt = vtp.tile([P, FH], f16, tag="vt", name=f"vtpre{g}")
                            nc.sync.dma_start(
                                out=vt,
                                in_=cc_v_out[c_idx, jlb * P:(jlb + 1) * P, 0:FH])
                            vt_pre.append(vt)

                        # q^T = Wh^T.T @ p^T -> [do, i], + bh; stays in SBUF
                        for dob in range(DB):
                            ps_t = psA.tile([P, NL], f32)
                            for db in range(DB):
                                for ih in range(2):
                                    nc.tensor.matmul(
                                        ps_t[:, ih * FH:(ih + 1) * FH],
                                        lhsT=WhT_sb[:, db, dob * P:(dob + 1) * P],
                                        rhs=pT_sb[:, db, ih * FH:(ih + 1) * FH],
                                        start=(db == 0), stop=(db == DB - 1))
                            nc.scalar.activation(out=qT[:, dob, :], in_=ps_t,
                                                 func=AF.Copy)

                    if phases == "A":
                        with tc.tile_pool(name="probe", bufs=2) as prb:
                            pe_t = prb.tile([P, NL], f32, tag="pe")
                            nc.vector.tensor_copy(out=pe_t, in_=qT[:, 0, :])
                            nc.sync.dma_start(out=out_h.ap()[0:P, :], in_=pe_t)
                        vtp_cm.__exit__(None, None, None)
                        continue

                    # -------- phase C: scores^T + local stats --------
                    ep_cm = tc.tile_pool(name="ep", bufs=1)
                    ep = ep_cm.__enter__()
                    E = ep.tile([P, NG, NL], f16)
                    with (
                        tc.tile_pool(name="ktp", bufs=2) as ktp,
                        tc.tile_pool(name="psC", bufs=3, space="PSUM") as psC,
                    ):
                        # next-shard K^T prefetch at current-shard start, on the
                        # scalar HWDGE ring: no Q7/SWDGE descriptor-gen cost and
                        # never serialized behind the collective waits that
                        # occupy the gpsimd queue.
                        kt_tiles = {0: kt_pre}
                        for c_idx in range(NCORES):
                            if c_idx + 1 < NCORES:
                                kt_n = ktp.tile([P, DB, JBL, P], f16, tag="kt")
                                for db in range(DB):
                                    nc.scalar.dma_start(
                                        out=kt_n[:, db, :, :].rearrange(
                                            "p a b -> p (a b)"),
                                        in_=cc_kt_out[c_idx + 1,
                                                      db * P:(db + 1) * P, :])
                                kt_tiles[c_idx + 1] = kt_n
                            kt_c = kt_tiles.pop(c_idx)
                            for jlb in range(JBL):
                                g = c_idx * JBL + jlb
                                ps_t = psC.tile([P, NL], f32, tag="sc")
                                for db in range(DB):
                                    for ih in range(2):
                                        nc.tensor.matmul(
                                            ps_t[:, ih * FH:(ih + 1) * FH],
                                            lhsT=kt_c[:, db, jlb, :],
                                            rhs=qT[:, db, ih * FH:(ih + 1) * FH],
                                            start=(db == 0), stop=(db == DB - 1))
                                nc.vector.tensor_reduce(
                                    out=stats[:, 0, g:g + 1], in_=ps_t,
                                    op=ALU.max, axis=AX.X, negate=True)
                                nc.scalar.activation(
                                    out=E[:, g, :], in_=ps_t, func=AF.Exp,
                                    bias=stats[:, 0, g:g + 1], scale=1.0,
                                    accum_out=stats[:, 1, g:g + 1])

                    with (
                        tc.tile_pool(name="ktp2", bufs=1) as _unused_ktp2,
                    ):
                        # stats AllGather + combine in two halves: the first
                        # half's AG/combine/E-scale hide under phase C's tail
                        NH = NG // 2
                        Mneg = lp.tile([P, NG], f32)
                        Ssum = lp.tile([P, NG], f32)
                        tmp = lp.tile([P, NG], f32)
                        diff = lp.tile([P, NG], f32)
                        alpha = lp.tile([P, NG], f32)
                        rec = lp.tile([P, NG], f32)
                        gath = [lp.tile([P, NCORES, 2, NH], f32, name=f"gath{h}")
                                for h in range(2)]
                        for h in range(2):
                            hs = slice(h * NH, (h + 1) * NH)
                            nc.sync.dma_start(out=cc_st_in[h][:, 0:NH],
                                              in_=stats[:, 0, hs])
                            nc.sync.dma_start(out=cc_st_in[h][:, NH:NG],
                                              in_=stats[:, 1, hs])
                            collective("AllGather", ALU.bypass,
                                       [cc_st_in[h]], [cc_st_out[h]])
                            nc.sync.dma_start(
                                out=gath[h].rearrange("p c a b -> p (c a b)"),
                                in_=cc_st_out[h].rearrange("c p x -> p c x"))
                            g_h = gath[h]
                            nc.vector.tensor_copy(out=Mneg[:, hs],
                                                  in_=g_h[:, 0, 0, :])
                            for c in range(1, NCORES):
                                nc.vector.tensor_tensor(out=Mneg[:, hs],
                                                        in0=Mneg[:, hs],
                                                        in1=g_h[:, c, 0, :],
                                                        op=ALU.min)
                            for c in range(NCORES):
                                nc.vector.tensor_sub(out=tmp[:, hs],
                                                     in0=Mneg[:, hs],
                                                     in1=g_h[:, c, 0, :])
                                nc.scalar.activation(out=tmp[:, hs],
                                                     in_=tmp[:, hs], func=AF.Exp)
                                nc.vector.tensor_mul(out=tmp[:, hs],
                                                     in0=tmp[:, hs],
                                                     in1=g_h[:, c, 1, :])
                                if c == 0:
                                    nc.vector.tensor_copy(out=Ssum[:, hs],
                                                          in_=tmp[:, hs])
                                else:
                                    nc.vector.tensor_add(out=Ssum[:, hs],
                                                         in0=Ssum[:, hs],
                                                         in1=tmp[:, hs])
                            # f = exp(Mneg - mneg_local) / Ssum, fold into E
                            nc.vector.tensor_sub(out=diff[:, hs], in0=Mneg[:, hs],
                                                 in1=stats[:, 0, hs])
                            nc.scalar.activation(out=alpha[:, hs], in_=diff[:, hs],
                                                 func=AF.Exp)
                            nc.vector.reciprocal(out=rec[:, hs], in_=Ssum[:, hs])
                            nc.vector.tensor_mul(out=f_sc[:, hs], in0=alpha[:, hs],
                                                 in1=rec[:, hs])
                            if h == 0:
                                # pre-scale the prefetched V tiles while ACT
                                # is otherwise free, so phase E's first MMs
                                # start the moment PSUM frees up
                                for g0 in range(VPRE):
                                    nc.scalar.activation(
                                        out=vt_pre[g0], in_=vt_pre[g0],
                                        func=AF.Copy,
                                        scale=f_sc[:, g0:g0 + 1])

                    if phases == "AC":
                        with tc.tile_pool(name="probe", bufs=2) as prb:
                            pe_t = prb.tile([P, NL], f32, tag="pe")
                            nc.vector.tensor_copy(out=pe_t, in_=E[:, NG - 1, :])
                            nc.sync.dma_start(out=out_h.ap()[0:P, :], in_=pe_t)
                            nc.sync.dma_start(out=out_h.ap()[P:2 * P, 0:NG],
                                              in_=f_sc)
                        ep_cm.__exit__(None, None, None)
                        vtp_cm.__exit__(None, None, None)
                        continue

                    # ---- phase E: out = E^T.T @ (f·V) + p, two D-half passes ----
                    # residual p enters PSUM via an identity matmul (fp16), so
                    # the post-loop tail is just ACT/DVE-alternating evictions
                    # that pipeline with the per-block residual matmuls.
                    with (
                        tc.tile_pool(name="prp", bufs=IB) as prp,
                        tc.tile_pool(name="osp", bufs=4) as osp,
                        tc.tile_pool(name="psE", bufs=1, space="PSUM") as psE,
                    ):
                        po = [psE.tile([P, FH], f32, tag=f"po{q_}",
                                       name=f"po{q_}")
                              for q_ in range(IB)]
                        for dvh in range(2):
                            pr = []
                            for q_ in range(IB):
                                prt = prp.tile([P, FH], f16, tag="pr")
                                nc.sync.dma_start(
                                    out=prt,
                                    in_=pres_h.ap()[q_ * P:(q_ + 1) * P,
                                                    dvh * FH:(dvh + 1) * FH])
                                pr.append(prt)
                            for g in range(NG):
                                c_idx, jlb = divmod(g, JBL)
                                if dvh == 0 and g < VPRE:
                                    vt = vt_pre[g]   # already f-scaled above
                                else:
                                    vt = vtp.tile([P, FH], f16, tag="vt")
                                    nc.sync.dma_start(
                                        out=vt,
                                        in_=cc_v_out[c_idx, jlb * P:(jlb + 1) * P,
                                                     dvh * FH:(dvh + 1) * FH])
                                    # fold softmax correction f into V rows
                                    # (per-key = per-partition scale); ACT is
                                    # idle in phase E
                                    nc.scalar.activation(
                                        out=vt, in_=vt, func=AF.Copy,
                                        scale=f_sc[:, g:g + 1])
                                for q_ in range(IB):
                                    nc.tensor.matmul(
                                        po[q_],
                                        lhsT=E[:, g, q_ * P:(q_ + 1) * P],
                                        rhs=vt,
                                        start=(g == 0), stop=False)
                            for q_ in range(IB):
                                nc.tensor.matmul(
                                    po[q_], lhsT=ident_sb, rhs=pr[q_],
                                    start=False, stop=True)
                                ot = osp.tile([P, FH], f32, tag="ot")
                                if q_ % 2 == 0:
                                    nc.scalar.activation(out=ot, in_=po[q_],
                                                         func=AF.Copy)
                                else:
                                    nc.vector.tensor_copy(out=ot, in_=po[q_])
                                nc.sync.dma_start(
                                    out=out_h.ap()[q_ * P:(q_ + 1) * P,
                                                   dvh * FH:(dvh + 1) * FH],
                                    in_=ot)
                    ep_cm.__exit__(None, None, None)
                    vtp_cm.__exit__(None, None, None)
            if spin_us:
                with tc.tile_critical():
                    for _ in range(spin_us):
                        nc.vector.nop(cycle_cnt=960)
    nc.compile()
    return nc


def prepare_in_maps(p, r, Wh, bh, Wl, bl, Wg, bg):
    f16 = np.float16
    f32 = np.float32
    WhT = np.ascontiguousarray(Wh.T).astype(f16)
    WlT = np.ascontiguousarray(Wl.T).astype(f16)
    WgT = np.ascontiguousarray(Wg.T).astype(f16)
    bl_r = np.ascontiguousarray(bl.astype(f32).reshape(DB, P).T)
    bg16 = bg.astype(f16).reshape(1, D)
    in_maps = []
    ident = np.eye(P, dtype=f16)
    for c in range(NCORES):
        sl = slice(c * NL, (c + 1) * NL)
        in_maps.append({
            "pT": np.ascontiguousarray(p[sl].T).astype(f16),
            "rT": np.ascontiguousarray(r[sl].T).astype(f16),
            "pres": np.ascontiguousarray(p[sl]).astype(f16),
            "WhT": WhT, "WlT": WlT, "WgT": WgT,
            "bl_r": bl_r, "bg16": bg16, "ones16": np.ones((1, P), f16),
            "ident": ident,
        })
    return in_maps


_NC_CACHE = {}


def kernel(p, r, Wh, bh, Wl, bl, Wg, bg):
    from concourse.bass_utils import run_bass_kernel_spmd

    p = np.asarray(p); r = np.asarray(r)
    in_maps = prepare_in_maps(p, r, np.asarray(Wh), np.asarray(bh),
                              np.asarray(Wl), np.asarray(bl),
                              np.asarray(Wg), np.asarray(bg))
    if 1 not in _NC_CACHE:
        _NC_CACHE[1] = build_nc(1)
    res = run_bass_kernel_spmd(_NC_CACHE[1], in_maps, list(range(NCORES)))
    out = np.concatenate([res.results[c]["out"] for c in range(NCORES)], axis=0)
    return out.astype(np.float32)

